# revision 8
# baseline (speedup 1.0000x reference)
"""CPCNet forward on 8 Trainium2 NeuronCores (Bass/Tile).  v5

Data-parallel over batch: each core does 16 of 128 batch elements
(embed GEMM -> GRU -> bilinear), parameters replicated, no collectives.

v5: X ships pre-cast bf16 with rows pre-ordered to the ET column order
(Xc/Xp: [s, b, k]; Xb: [nb, s, b, k]).  The embed's X^T is produced two
ways in parallel:
  - J_X=34 k-chunks per Xb block stream in ALREADY TRANSPOSED via the
    DMA xbar (HWDGE dma_start(transpose=True), DRAM->SBUF, 8 dedicated
    SDMA engines E80-87) -- zero PE/DVE/ACT cost for those chunks;
  - the remaining chunks load naturally via gpsimd SWDGE (16 engines)
    and go through the PE transpose + DVE/ACT evacuation pipeline.
The two DMA paths use disjoint SDMA engine sets so they overlap; the
PE transpose load drops ~55%, putting every engine under ~170us.
"""

import numpy as np

import concourse.bacc as bacc
import concourse.mybir as mybir
import concourse.tile as tile
from concourse.bass_utils import run_bass_kernel_spmd

N_CORES = 8
BC = 16          # batch per core
NE = 16          # context windows (gru seq len)
NB = 10          # negative samples
CT = 8400        # flattened window (21*400)
E = 100          # embed dim == gru hidden
ROWS = BC * NE * (2 + NB)   # 3072 rows per core
NBLK = ROWS // 512          # 6 blocks of 512 rows
NCHUNK = 66                 # ceil(8400/128); last chunk is 80 wide
J_X = 34                    # leading k-chunks per Xb block via DMA xbar
# block 0 (Xc+Xp): all 66 chunks through the PE pipeline, 4 slabs
SLABS_B0 = [(0, 2304), (2304, 2048), (4352, 2048), (6400, 2000)]
# blocks 1-5 (Xb): PE pipeline covers k >= J_X*128 only, 2 slabs
SLABS_XB = [(4352, 2048), (6400, 2000)]
SLABW = 2304                # natural-load tile width (prefix used)

F32 = mybir.dt.float32
BF16 = mybir.dt.bfloat16


def _block_src(Xc, Xp, Xb, blk, st, k0, kw):
    """DRAM source AP for 128-row subtile `st` of 512-row block `blk`,
    k-range [k0, k0+kw).  Row order (s-major, b-minor) matches the
    DRAM layout directly now."""
    sh = st % 2
    if blk == 0:
        base = Xc if st < 2 else Xp
        return base[sh * 8:(sh + 1) * 8, :, k0:k0 + kw]
    nb = 2 * (blk - 1) + st // 2
    return Xb[nb, sh * 8:(sh + 1) * 8, :, k0:k0 + kw]


def _emit(nc, tc, ctx):
    Xc = nc.dram_tensor("Xc", [NE, BC, CT], BF16, kind="ExternalInput").ap()
    Xp = nc.dram_tensor("Xp", [NE, BC, CT], BF16, kind="ExternalInput").ap()
    Xb = nc.dram_tensor("Xb", [NB, NE, BC, CT], BF16, kind="ExternalInput").ap()
    Wemb = nc.dram_tensor("Wemb", [128, NCHUNK * E], BF16,
                          kind="ExternalInput").ap()
    bemb = nc.dram_tensor("bemb", [E, 1], F32, kind="ExternalInput").ap()
    WihT = nc.dram_tensor("WihT", [E, 300], F32, kind="ExternalInput").ap()
    WhhT = nc.dram_tensor("WhhT", [E, 300], F32, kind="ExternalInput").ap()
    bias4 = nc.dram_tensor("bias4", [E, 4], F32, kind="ExternalInput").ap()
    Wbil = nc.dram_tensor("Wbil", [E, NE * E], F32, kind="ExternalInput").ap()
    ident = nc.dram_tensor("ident", [128, 128], BF16, kind="ExternalInput").ap()
    ones = nc.dram_tensor("ones", [E, 1], mybir.dt.float32r,
                          kind="ExternalInput").ap()
    out_d = nc.dram_tensor("out", [1, NE * BC * (NB + 1)], F32,
                           kind="ExternalOutput").ap()

    P = ctx.enter_context  # pools

    const = P(tc.tile_pool(name="const", bufs=1))
    xnat = P(tc.tile_pool(name="xnat", bufs=3))
    xbarp = P(tc.tile_pool(name="xbar", bufs=2))
    xtp = P(tc.tile_pool(name="xt", bufs=4))
    psT = P(tc.tile_pool(name="psT", bufs=3, space="PSUM"))
    psE = P(tc.tile_pool(name="psE", bufs=1, space="PSUM"))
    psS = P(tc.tile_pool(name="psS", bufs=1, space="PSUM"))
    small = P(tc.tile_pool(name="small", bufs=2))

    # ---- persistent SBUF ----
    id_sb = const.tile([128, 128], BF16)
    nc.sync.dma_start(id_sb[:], ident[:])
    W_sb = const.tile([128, NCHUNK * E], BF16)
    nc.sync.dma_start(W_sb[:], Wemb[:])
    bemb_sb = const.tile([E, 1], F32)
    nc.scalar.dma_start(bemb_sb[:], bemb[:])
    WihT_sb = const.tile([E, 300], F32)
    nc.scalar.dma_start(WihT_sb[:], WihT[:])
    WhhT_sb = const.tile([E, 300], F32)
    nc.scalar.dma_start(WhhT_sb[:], WhhT[:])
    bias4_sb = const.tile([E, 4], F32)
    nc.scalar.dma_start(bias4_sb[:], bias4[:])
    Wbil_sb = const.tile([E, NE * E], F32)
    nc.scalar.dma_start(Wbil_sb[:], Wbil[:])
    ones_sb = const.tile([E, 1], mybir.dt.float32r)
    nc.scalar.dma_start(ones_sb[:], ones[:])

    ET = const.tile([E, ROWS], F32)                # all embeddings, transposed
    gi_sb = const.tile([E, NE * 3 * BC], F32)      # preacts, [s][r|z|n] blocks
    h = const.tile([E, BC], F32)                   # GRU hidden state (h^T)
    tmp_all = const.tile([E, NE * BC * (NB + 1)], mybir.dt.float32r)
    out_sb = const.tile([1, NE * BC * (NB + 1)], F32)

    A_sb = const.tile([E, NE * BC], F32)           # bilinear A, persisted
    gi_v = gi_sb.rearrange("e (s g b) -> e s g b", s=NE, g=3)
    tmp_v = tmp_all.rearrange("e (s b p) -> e s b p", s=NE, b=BC)
    Eb_v = ET[:, 512:ROWS].rearrange("e (nb s b) -> e nb s b", nb=NB, s=NE)

    def bil_part(s0, s1):
        for s in range(s0, s1):
            Ap = psS.tile([E, BC], F32, tag="bilA", name="Ap", bufs=2)
            nc.tensor.matmul(Ap[:, :], Wbil_sb[:, s * E:(s + 1) * E], h[:],
                             start=True, stop=True)
            nc.scalar.copy(A_sb[:, s * BC:(s + 1) * BC], Ap[:])
            nc.vector.tensor_mul(tmp_v[:, s, :, 0],
                                 ET[:, NE * BC + s * BC: NE * BC + (s + 1) * BC],
                                 Ap[:])
            nc.vector.tensor_mul(
                tmp_v[:, s, :, 1:9].rearrange("e b p -> e p b"),
                Eb_v[:, 0:8, s, :],
                Ap[:].unsqueeze(1).broadcast_to([E, 8, BC]))

    def gru_init():
        nc.vector.memset(h[:], 0.0)
        for g in range(3):
            gp = psS.tile([E, NE * BC], F32, tag="sp0", name="gp")
            nc.tensor.matmul(gp[:, :], WihT_sb[:, g * E:(g + 1) * E],
                             ET[:, 0:NE * BC], start=True, stop=True)
            nc.scalar.add(gi_v[:, :, g, :],
                          gp.rearrange("e (s b) -> e s b", s=NE),
                          bias4_sb[:, g:g + 1])

    def gru_step(s):
        c0 = s * 3 * BC
        gh = psS.tile([E, 3 * BC], F32, tag="sp1", name="gh")
        for g in range(3):
            nc.tensor.matmul(gh[:, g * BC:(g + 1) * BC],
                             WhhT_sb[:, g * E:(g + 1) * E], h[:],
                             start=True, stop=True)
        ghs = small.tile([E, 3 * BC], F32, tag="ghs", name="ghs")
        nc.vector.tensor_copy(ghs[:], gh[:])
        rzt = small.tile([E, 2 * BC], F32, tag="rzt", name="rzt")
        nc.gpsimd.tensor_add(rzt[:], ghs[:, 0:2 * BC], gi_sb[:, c0:c0 + 2 * BC])
        rz = small.tile([E, 2 * BC], F32, tag="rz", name="rz")
        nc.scalar.activation(rz[:], rzt[:],
                             mybir.ActivationFunctionType.Sigmoid)
        hn = small.tile([E, BC], F32, tag="hn", name="hn")
        nc.gpsimd.tensor_scalar_add(hn[:], ghs[:, 2 * BC:3 * BC],
                                    bias4_sb[:, 3:4])  # gh_n + b_hn
        t1 = small.tile([E, BC], F32, tag="t1", name="t1")
        nc.gpsimd.tensor_mul(t1[:], rz[:, 0:BC], hn[:])
        t2 = small.tile([E, BC], F32, tag="t2", name="t2")
        nc.gpsimd.tensor_add(t2[:], t1[:], gi_sb[:, c0 + 2 * BC:c0 + 3 * BC])
        n = small.tile([E, BC], F32, tag="n", name="n")
        nc.scalar.activation(n[:], t2[:], mybir.ActivationFunctionType.Tanh)
        d = small.tile([E, BC], F32, tag="d", name="d")
        nc.gpsimd.tensor_sub(d[:], h[:], n[:])
        zd = small.tile([E, BC], F32, tag="zd", name="zd")
        nc.gpsimd.tensor_mul(zd[:], rz[:, BC:2 * BC], d[:])
        nc.gpsimd.tensor_add(h[:], n[:], zd[:])    # h = n + z*(h-n)

    def xbar_issue(blk):
        """Issue the J_X transposed chunk loads for Xb block `blk`
        (1..5): DRAM [512 rows, 128 k] -> SBUF [128 k, 512 rows]."""
        nb0 = 2 * (blk - 1)
        src = Xb[nb0:nb0 + 2].rearrange("nb s b k -> (nb s b) k")
        tiles = []
        for j in range(J_X):
            xbt = xbarp.tile([128, 512], BF16, tag=f"xb{j}", name=f"xb{j}")
            nc.sync.dma_start(xbt[:], src[:, j * 128:(j + 1) * 128],
                              transpose=True)
            tiles.append(xbt)
        return tiles

    # prime the xbar pipeline for block 1 before block 0's compute
    xb_tiles = xbar_issue(1)

    # ---- embed: 6 blocks of 512 rows; GRU interleaved after block 0 ----
    for blk in range(NBLK):
        et = psE.tile([E, 512], F32)
        nmm = 0
        if blk >= 1:
            # xbar-loaded chunks: straight accumulating matmuls, no
            # transpose/evac; dense PE work while the slab DMAs land
            cur = xb_tiles
            if blk < NBLK - 1:
                xb_tiles = xbar_issue(blk + 1)
            for j in range(J_X):
                nc.tensor.matmul(
                    et[:, :], W_sb[:, j * E:(j + 1) * E], cur[j][:],
                    start=(nmm == 0), stop=False, skip_group_check=True)
                nmm += 1
        slabs = SLABS_B0 if blk == 0 else SLABS_XB
        for si, (k0, kw) in enumerate(slabs):
            xs = [xnat.tile([128, SLABW], BF16, tag=f"xn{st}", name=f"xn{st}")
                  for st in range(4)]
            for st in range(4):
                nc.gpsimd.dma_start(xs[st][:, 0:kw],
                                    _block_src(Xc, Xp, Xb, blk, st, k0, kw))
            nj = kw // 128 + (1 if kw % 128 else 0)
            assert nj % 2 == 0
            jbase = k0 // 128
            for jp in range(nj // 2):
                # interleave serial GRU / bilinear work inside the stream
                if 1 <= blk <= 4 and jp in (0, 4):
                    gru_step(4 * (blk - 1) + 2 * si + (0 if jp == 0 else 1))
                elif blk == 5 and jp in (0, 4):
                    q = 2 * si + (0 if jp == 0 else 1)
                    bil_part(4 * q, 4 * q + 4)
                pt = psT.tile([128, 1024], BF16)
                kjs = []
                for u in range(2):
                    j = jp * 2 + u
                    kj = min(128, CT - (k0 + j * 128))
                    kjs.append(kj)
                    for st in range(4):
                        nc.tensor.transpose(
                            pt[0:kj, u * 512 + st * 128:u * 512 + (st + 1) * 128],
                            xs[st][:, j * 128:j * 128 + kj],
                            id_sb[:])
                xt = xtp.tile([128, 1024], BF16)
                if kjs[1] == 128:
                    nc.vector.tensor_copy(xt[:, 0:640], pt[:, 0:640])
                    nc.scalar.copy(xt[:, 640:1024], pt[:, 640:1024])
                else:  # last pair: u=1 chunk only has kjs[1] valid rows
                    nc.vector.tensor_copy(xt[:, 0:512], pt[:, 0:512])
                    nc.scalar.copy(xt[0:kjs[1], 512:1024], pt[0:kjs[1], 512:1024])
                for u in range(2):
                    jg = jbase + jp * 2 + u
                    nc.tensor.matmul(
                        et[:, :],
                        W_sb[0:kjs[u], jg * E:(jg + 1) * E],
                        xt[0:kjs[u], u * 512:u * 512 + 512],
                        start=(nmm == 0), stop=(nmm == NCHUNK - 1),
                        skip_group_check=True)
                    nmm += 1
        # bias + evacuate to ET
        nc.scalar.add(ET[:, blk * 512:(blk + 1) * 512], et[:, :],
                      bemb_sb[:, 0:1])
        # gi preacts as soon as block 0 (Ec) is done
        if blk == 0:
            gru_init()

    # ---- bilinear tail: only the nb8-9 (block 5) products remain ----
    for s in range(NE):
        nc.vector.tensor_mul(
            tmp_v[:, s, :, 9:NB + 1].rearrange("e b p -> e p b"),
            Eb_v[:, 8:10, s, :],
            A_sb[:, s * BC:(s + 1) * BC].unsqueeze(1).broadcast_to([E, 2, BC]))
    TOT = NE * BC * (NB + 1)
    for c0 in range(0, TOT, 512):
        w = min(512, TOT - c0)
        rp = psS.tile([1, 512], F32, tag="sp1")
        nc.tensor.matmul(rp[0:1, 0:w], ones_sb[:, 0:1], tmp_all[:, c0:c0 + w],
                         start=True, stop=True)
        nc.scalar.copy(out_sb[:, c0:c0 + w], rp[0:1, 0:w])
    nc.sync.dma_start(out_d[:], out_sb[:])


def build():
    import contextlib
    nc = bacc.Bacc("TRN2", target_bir_lowering=False, debug=False,
                   enable_asserts=False, num_devices=N_CORES)
    with tile.TileContext(nc) as tc:
        with contextlib.ExitStack() as ctx:
            _emit(nc, tc, ctx)
    nc.compile()
    return nc


_NC = None


def make_in_maps(Xc, Xp, Xb, W_embed, b_embed, W_ih, W_hh, b_ih, b_hh, W_bil):
    import ml_dtypes
    B = Xc.shape[0]
    Xc_b = np.asarray(Xc, np.float32).reshape(B, NE, CT).astype(ml_dtypes.bfloat16)
    Xp_b = np.asarray(Xp, np.float32).reshape(B, NE, CT).astype(ml_dtypes.bfloat16)
    Xb_b = np.asarray(Xb, np.float32).reshape(B, NE, NB, CT).astype(ml_dtypes.bfloat16)

    W_embed = np.ascontiguousarray(W_embed, np.float32)
    W_ch = np.zeros((128, NCHUNK * E), np.float32)
    for j in range(NCHUNK):
        kj = min(128, CT - j * 128)
        W_ch[:kj, j * E:(j + 1) * E] = W_embed[j * 128:j * 128 + kj]
    W_ch = W_ch.astype(ml_dtypes.bfloat16)
    bemb = np.ascontiguousarray(b_embed, np.float32).reshape(E, 1)
    WihT = np.ascontiguousarray(W_ih.T, np.float32)          # [100, 300]
    WhhT = np.ascontiguousarray(W_hh.T, np.float32)
    bias4 = np.stack([b_ih[0:E] + b_hh[0:E],
                      b_ih[E:2 * E] + b_hh[E:2 * E],
                      b_ih[2 * E:3 * E],
                      b_hh[2 * E:3 * E]], axis=1).astype(np.float32)
    Wbil_r = np.ascontiguousarray(
        np.transpose(W_bil, (1, 0, 2)).reshape(E, NE * E), np.float32)
    ident = np.eye(128).astype(ml_dtypes.bfloat16)
    ones = np.ones((E, 1), np.float32)

    shared = dict(Wemb=W_ch, bemb=bemb, WihT=WihT, WhhT=WhhT,
                  bias4=bias4, Wbil=Wbil_r, ident=ident, ones=ones)
    in_maps = []
    for c in range(N_CORES):
        sl = slice(c * BC, (c + 1) * BC)
        in_maps.append(dict(
            Xc=np.ascontiguousarray(Xc_b[sl].transpose(1, 0, 2)),
            Xp=np.ascontiguousarray(Xp_b[sl].transpose(1, 0, 2)),
            Xb=np.ascontiguousarray(Xb_b[sl].transpose(2, 1, 0, 3)),
            **shared))
    return in_maps


def gather(results):
    outs = []
    for c in range(N_CORES):
        o = results[c]["out"].reshape(NE, BC, NB + 1)       # [s, b, p]
        outs.append(np.transpose(o, (1, 0, 2)))             # [b, s, p]
    return np.concatenate(outs, axis=0).astype(np.float32)  # [128, 16, 11]


def kernel(Xc, Xp, Xb, W_embed, b_embed, W_ih, W_hh, b_ih, b_hh, W_bil):
    global _NC
    if _NC is None:
        _NC = build()
    in_maps = make_in_maps(Xc, Xp, Xb, W_embed, b_embed, W_ih, W_hh,
                           b_ih, b_hh, W_bil)
    res = run_bass_kernel_spmd(_NC, in_maps, core_ids=list(range(N_CORES)))
    return gather(res.results)


# revision 10
# speedup vs baseline: 2.0637x; 2.0637x over previous
"""CPCNet forward on 8 Trainium2 NeuronCores (Bass/Tile).  v6

Data-parallel over batch: each core does 16 of 128 batch elements
(embed GEMM -> GRU over 16 context windows -> bilinear scoring),
parameters replicated, no collectives.

Input staging (host, inside kernel(), like the W_embed pre-chunk/cast):
X ships to device DRAM as XT = X^T in bf16 -- one [8448, 3072] tensor
per core, rows = the flattened C*T window dim (zero-padded 8400->8448 to
a whole number of 128-chunks), cols = the 3072 per-core windows in ET
column order (Xc 256 | Xp 256 | Xb nb-major 2560).

The device kernel is then a pure streaming GEMM at the bf16 HBM
roofline: 3 passes over k, each covering a 1024-column block-pair.
Per k-chunk one [128, 1024] tile loads (DMAs alternate between gpsimd
SWDGE/16 engines and sync HWDGE/8 engines so issue queues and SDMA
engine sets both stay parallel) and feeds two accumulating matmuls
(W chunk stationary).  No on-chip transposes, no PSUM evacuation
pipeline -- PE/DVE/ACT all run far below the DMA time.

GRU steps hide inside pass 1, bilinear A/products inside pass 2 (the
products that need blocks 4-5 run as a short tail with the float32r
ones-matmul reduction).
"""

import numpy as np

import concourse.bacc as bacc
import concourse.mybir as mybir
import concourse.tile as tile
from concourse.bass_utils import run_bass_kernel_spmd

N_CORES = 8
BC = 16          # batch per core
NE = 16          # context windows (gru seq len)
NB = 10          # negative samples
CT = 8400        # flattened window (21*400)
E = 100          # embed dim == gru hidden
ROWS = BC * NE * (2 + NB)   # 3072 rows per core
NCHUNK = 66                 # 8448 / 128 k-chunks (last 48 rows zero-pad)
CTP = NCHUNK * 128          # 8448

F32 = mybir.dt.float32
BF16 = mybir.dt.bfloat16


def _emit(nc, tc, ctx):
    XT = nc.dram_tensor("XT", [CTP, ROWS], BF16, kind="ExternalInput").ap()
    Wemb = nc.dram_tensor("Wemb", [128, NCHUNK * E], BF16,
                          kind="ExternalInput").ap()
    bemb = nc.dram_tensor("bemb", [E, 1], F32, kind="ExternalInput").ap()
    WihT = nc.dram_tensor("WihT", [E, 300], F32, kind="ExternalInput").ap()
    WhhT = nc.dram_tensor("WhhT", [E, 300], F32, kind="ExternalInput").ap()
    bias4 = nc.dram_tensor("bias4", [E, 4], F32, kind="ExternalInput").ap()
    Wbil = nc.dram_tensor("Wbil", [E, NE * E], F32, kind="ExternalInput").ap()
    ones = nc.dram_tensor("ones", [E, 1], mybir.dt.float32r,
                          kind="ExternalInput").ap()
    out_d = nc.dram_tensor("out", [1, NE * BC * (NB + 1)], F32,
                           kind="ExternalOutput").ap()

    P = ctx.enter_context  # pools

    const = P(tc.tile_pool(name="const", bufs=1))
    xtp = P(tc.tile_pool(name="xt", bufs=10))
    psE = P(tc.tile_pool(name="psE", bufs=2, space="PSUM"))
    psS = P(tc.tile_pool(name="psS", bufs=1, space="PSUM"))
    small = P(tc.tile_pool(name="small", bufs=2))

    # ---- persistent SBUF ----
    W_sb = const.tile([128, NCHUNK * E], BF16)
    nc.sync.dma_start(W_sb[:], Wemb[:])
    bemb_sb = const.tile([E, 1], F32)
    nc.scalar.dma_start(bemb_sb[:], bemb[:])
    WihT_sb = const.tile([E, 300], F32)
    nc.scalar.dma_start(WihT_sb[:], WihT[:])
    WhhT_sb = const.tile([E, 300], F32)
    nc.scalar.dma_start(WhhT_sb[:], WhhT[:])
    bias4_sb = const.tile([E, 4], F32)
    nc.scalar.dma_start(bias4_sb[:], bias4[:])
    Wbil_sb = const.tile([E, NE * E], F32)
    nc.scalar.dma_start(Wbil_sb[:], Wbil[:])
    ones_sb = const.tile([E, 1], mybir.dt.float32r)
    nc.scalar.dma_start(ones_sb[:], ones[:])

    ET = const.tile([E, ROWS], F32)                # all embeddings, transposed
    gi_sb = const.tile([E, NE * 3 * BC], F32)      # preacts, [s][r|z|n] blocks
    h = const.tile([E, BC], F32)                   # GRU hidden state (h^T)
    tmp_all = const.tile([E, NE * BC * (NB + 1)], mybir.dt.float32r)
    out_sb = const.tile([1, NE * BC * (NB + 1)], F32)

    A_sb = const.tile([E, NE * BC], F32)           # bilinear A, persisted
    gi_v = gi_sb.rearrange("e (s g b) -> e s g b", s=NE, g=3)
    tmp_v = tmp_all.rearrange("e (s b p) -> e s b p", s=NE, b=BC)
    Eb_v = ET[:, 512:ROWS].rearrange("e (nb s b) -> e nb s b", nb=NB, s=NE)

    def bil_part(s0, s1):
        # A_s = W_bil[s].T @ h^T plus the Ep and nb0-5 score products
        # (blocks 0-3, all evacuated by end of pass 1) -- spread over
        # pass 2; the nb6-9 products run as the tail.
        for s in range(s0, s1):
            Ap = psS.tile([E, BC], F32, tag="bilA", name="Ap", bufs=2)
            nc.tensor.matmul(Ap[:, :], Wbil_sb[:, s * E:(s + 1) * E], h[:],
                             start=True, stop=True)
            nc.scalar.copy(A_sb[:, s * BC:(s + 1) * BC], Ap[:])
            nc.vector.tensor_mul(tmp_v[:, s, :, 0],
                                 ET[:, NE * BC + s * BC: NE * BC + (s + 1) * BC],
                                 Ap[:])
            nc.vector.tensor_mul(
                tmp_v[:, s, :, 1:7].rearrange("e b p -> e p b"),
                Eb_v[:, 0:6, s, :],
                Ap[:].unsqueeze(1).broadcast_to([E, 6, BC]))

    def gru_init():
        nc.vector.memset(h[:], 0.0)
        for g in range(3):
            gp = psS.tile([E, NE * BC], F32, tag="sp0", name="gp")
            nc.tensor.matmul(gp[:, :], WihT_sb[:, g * E:(g + 1) * E],
                             ET[:, 0:NE * BC], start=True, stop=True)
            nc.scalar.add(gi_v[:, :, g, :],
                          gp.rearrange("e (s b) -> e s b", s=NE),
                          bias4_sb[:, g:g + 1])

    def gru_step(s):
        # elementwise on the idle GpSimd, sigmoid/tanh on ACT; DVE only
        # evacuates gh so nothing queues behind the serial chain
        c0 = s * 3 * BC
        gh = psS.tile([E, 3 * BC], F32, tag="sp1", name="gh")
        for g in range(3):
            nc.tensor.matmul(gh[:, g * BC:(g + 1) * BC],
                             WhhT_sb[:, g * E:(g + 1) * E], h[:],
                             start=True, stop=True)
        ghs = small.tile([E, 3 * BC], F32, tag="ghs", name="ghs")
        nc.vector.tensor_copy(ghs[:], gh[:])
        rzt = small.tile([E, 2 * BC], F32, tag="rzt", name="rzt")
        nc.gpsimd.tensor_add(rzt[:], ghs[:, 0:2 * BC], gi_sb[:, c0:c0 + 2 * BC])
        rz = small.tile([E, 2 * BC], F32, tag="rz", name="rz")
        nc.scalar.activation(rz[:], rzt[:],
                             mybir.ActivationFunctionType.Sigmoid)
        hn = small.tile([E, BC], F32, tag="hn", name="hn")
        nc.gpsimd.tensor_scalar_add(hn[:], ghs[:, 2 * BC:3 * BC],
                                    bias4_sb[:, 3:4])  # gh_n + b_hn
        t1 = small.tile([E, BC], F32, tag="t1", name="t1")
        nc.gpsimd.tensor_mul(t1[:], rz[:, 0:BC], hn[:])
        t2 = small.tile([E, BC], F32, tag="t2", name="t2")
        nc.gpsimd.tensor_add(t2[:], t1[:], gi_sb[:, c0 + 2 * BC:c0 + 3 * BC])
        n = small.tile([E, BC], F32, tag="n", name="n")
        nc.scalar.activation(n[:], t2[:], mybir.ActivationFunctionType.Tanh)
        d = small.tile([E, BC], F32, tag="d", name="d")
        nc.gpsimd.tensor_sub(d[:], h[:], n[:])
        zd = small.tile([E, BC], F32, tag="zd", name="zd")
        nc.gpsimd.tensor_mul(zd[:], rz[:, BC:2 * BC], d[:])
        nc.gpsimd.tensor_add(h[:], n[:], zd[:])    # h = n + z*(h-n)

    # ---- embed: 3 passes over k, each a 1024-column block-pair ----
    for p in range(3):
        et0 = psE.tile([E, 512], F32, tag="et0", name="et0")
        et1 = psE.tile([E, 512], F32, tag="et1", name="et1")
        c0 = p * 1024
        for j in range(NCHUNK):
            # interleaved serial work: GRU in pass 1, bilinear in pass 2
            if p == 1 and j % 4 == 0 and j < 64:
                gru_step(j // 4)
            elif p == 2 and j % 16 == 0 and j < 64:
                q = j // 16
                bil_part(4 * q, 4 * q + 4)
            xt = xtp.tile([128, 1024], BF16, name="xt")
            eng = nc.gpsimd if j % 2 == 0 else nc.sync
            eng.dma_start(xt[:], XT[j * 128:(j + 1) * 128, c0:c0 + 1024])
            nc.tensor.matmul(et0[:, :], W_sb[:, j * E:(j + 1) * E],
                             xt[:, 0:512],
                             start=(j == 0), stop=(j == NCHUNK - 1),
                             skip_group_check=True)
            nc.tensor.matmul(et1[:, :], W_sb[:, j * E:(j + 1) * E],
                             xt[:, 512:1024],
                             start=(j == 0), stop=(j == NCHUNK - 1),
                             skip_group_check=True)
        nc.scalar.add(ET[:, c0:c0 + 512], et0[:, :], bemb_sb[:, 0:1])
        nc.scalar.add(ET[:, c0 + 512:c0 + 1024], et1[:, :], bemb_sb[:, 0:1])
        # gi preacts as soon as block 0 (Ec) is done
        if p == 0:
            gru_init()

    # ---- tail: nb6-9 products (blocks 4-5) + ones-matmul reduction ----
    for s in range(NE):
        nc.vector.tensor_mul(
            tmp_v[:, s, :, 7:NB + 1].rearrange("e b p -> e p b"),
            Eb_v[:, 6:10, s, :],
            A_sb[:, s * BC:(s + 1) * BC].unsqueeze(1).broadcast_to([E, 4, BC]))
    TOT = NE * BC * (NB + 1)
    for cc in range(0, TOT, 512):
        w = min(512, TOT - cc)
        rp = psS.tile([1, 512], F32, tag="sp1")
        nc.tensor.matmul(rp[0:1, 0:w], ones_sb[:, 0:1], tmp_all[:, cc:cc + w],
                         start=True, stop=True)
        nc.scalar.copy(out_sb[:, cc:cc + w], rp[0:1, 0:w])
    nc.sync.dma_start(out_d[:], out_sb[:])


def build():
    import contextlib
    nc = bacc.Bacc("TRN2", target_bir_lowering=False, debug=False,
                   enable_asserts=False, num_devices=N_CORES)
    with tile.TileContext(nc) as tc:
        with contextlib.ExitStack() as ctx:
            _emit(nc, tc, ctx)
    nc.compile()
    return nc


_NC = None


def make_in_maps(Xc, Xp, Xb, W_embed, b_embed, W_ih, W_hh, b_ih, b_hh, W_bil):
    import ml_dtypes
    B = Xc.shape[0]
    BF = ml_dtypes.bfloat16
    Xc_b = np.asarray(Xc, np.float32).reshape(B, NE, CT).astype(BF)
    Xp_b = np.asarray(Xp, np.float32).reshape(B, NE, CT).astype(BF)
    Xb_b = np.asarray(Xb, np.float32).reshape(B, NE, NB, CT).astype(BF)

    W_embed = np.ascontiguousarray(W_embed, np.float32)
    W_ch = np.zeros((128, NCHUNK * E), np.float32)
    for j in range(NCHUNK):
        kj = min(128, CT - j * 128)
        W_ch[:kj, j * E:(j + 1) * E] = W_embed[j * 128:j * 128 + kj]
    W_ch = W_ch.astype(BF)
    bemb = np.ascontiguousarray(b_embed, np.float32).reshape(E, 1)
    WihT = np.ascontiguousarray(W_ih.T, np.float32)          # [100, 300]
    WhhT = np.ascontiguousarray(W_hh.T, np.float32)
    bias4 = np.stack([b_ih[0:E] + b_hh[0:E],
                      b_ih[E:2 * E] + b_hh[E:2 * E],
                      b_ih[2 * E:3 * E],
                      b_hh[2 * E:3 * E]], axis=1).astype(np.float32)
    Wbil_r = np.ascontiguousarray(
        np.transpose(W_bil, (1, 0, 2)).reshape(E, NE * E), np.float32)
    ones = np.ones((E, 1), np.float32)

    shared = dict(Wemb=W_ch, bemb=bemb, WihT=WihT, WhhT=WhhT,
                  bias4=bias4, Wbil=Wbil_r, ones=ones)
    in_maps = []
    for c in range(N_CORES):
        sl = slice(c * BC, (c + 1) * BC)
        # rows in ET column order: Xc (s,b) | Xp (s,b) | Xb (nb,s,b)
        A = np.empty((ROWS, CT), BF)
        A[0:256] = Xc_b[sl].transpose(1, 0, 2).reshape(256, CT)
        A[256:512] = Xp_b[sl].transpose(1, 0, 2).reshape(256, CT)
        A[512:] = Xb_b[sl].transpose(2, 1, 0, 3).reshape(2560, CT)
        XTc = np.zeros((CTP, ROWS), BF)
        XTc[0:CT] = A.T
        in_maps.append(dict(XT=XTc, **shared))
    return in_maps


def gather(results):
    outs = []
    for c in range(N_CORES):
        o = results[c]["out"].reshape(NE, BC, NB + 1)       # [s, b, p]
        outs.append(np.transpose(o, (1, 0, 2)))             # [b, s, p]
    return np.concatenate(outs, axis=0).astype(np.float32)  # [128, 16, 11]


def kernel(Xc, Xp, Xb, W_embed, b_embed, W_ih, W_hh, b_ih, b_hh, W_bil):
    global _NC
    if _NC is None:
        _NC = build()
    in_maps = make_in_maps(Xc, Xp, Xb, W_embed, b_embed, W_ih, W_hh,
                           b_ih, b_hh, W_bil)
    res = run_bass_kernel_spmd(_NC, in_maps, core_ids=list(range(N_CORES)))
    return gather(res.results)


# revision 11
# speedup vs baseline: 2.6763x; 1.2969x over previous
"""CPCNet forward on 8 Trainium2 NeuronCores (Bass/Tile).  v7

Data-parallel over batch: each core does 16 of 128 batch elements
(embed GEMM -> GRU over 16 context windows -> bilinear scoring),
parameters replicated, no collectives.

Input staging (host, inside kernel(), like the W_embed pre-chunk/cast):
X ships to device DRAM transposed AND pass-packed in bf16.  For each of
3 column passes p (a 1024-column block-pair of the 3072 per-core
windows), XTp[q, j*1024+c] = X^T[k=j*128+q, row=p*1024+c]: partition q
holds k-chunk j's row contiguously, so a 4-chunk [128, 4096] tile is ONE
1-MB DMA with 8-KB-contiguous per-partition segments -- near-line-rate
HBM streaming (17 DMAs per pass, alternating gpsimd SWDGE / sync HWDGE
issue queues).

Device kernel = streaming GEMM at the bf16 HBM roofline: per chunk two
accumulating matmuls (W chunk stationary) into the pass's two PSUM
banks; bias-evac to ET[100, 3072] per pass.  No on-chip transposes.

The serial GRU chain runs on DVE+ACT only (gpsimd does nothing but DMA
issue, so the chain never queues behind buffer-full DMA waits -- in v6
that stretched the 16 steps to 150us and serialized them after the
stream).  GRU hides in pass 1, bilinear A+products in pass 2; the tail
is just the nb6-9 products and the float32r ones-matmul reduction.
"""

import numpy as np

import concourse.bacc as bacc
import concourse.mybir as mybir
import concourse.tile as tile
from concourse.bass_utils import run_bass_kernel_spmd

N_CORES = 8
BC = 16          # batch per core
NE = 16          # context windows (gru seq len)
NB = 10          # negative samples
CT = 8400        # flattened window (21*400)
E = 100          # embed dim == gru hidden
ROWS = BC * NE * (2 + NB)   # 3072 rows per core
NCHUNK = 66                 # 8448 / 128 k-chunks (last 48 rows zero-pad)
CTP = NCHUNK * 128          # 8448
NT = 17                     # 4-chunk tiles per pass (last tile: 2 chunks)

F32 = mybir.dt.float32
BF16 = mybir.dt.bfloat16


def _emit(nc, tc, ctx):
    XTs = [nc.dram_tensor(f"XT{p}", [128, NCHUNK * 1024], BF16,
                          kind="ExternalInput").ap() for p in range(3)]
    Wemb = nc.dram_tensor("Wemb", [128, NCHUNK * E], BF16,
                          kind="ExternalInput").ap()
    bemb = nc.dram_tensor("bemb", [E, 1], F32, kind="ExternalInput").ap()
    WihT = nc.dram_tensor("WihT", [E, 300], F32, kind="ExternalInput").ap()
    WhhT = nc.dram_tensor("WhhT", [E, 300], F32, kind="ExternalInput").ap()
    bias4 = nc.dram_tensor("bias4", [E, 4], F32, kind="ExternalInput").ap()
    Wbil = nc.dram_tensor("Wbil", [E, NE * E], F32, kind="ExternalInput").ap()
    ones = nc.dram_tensor("ones", [E, 1], mybir.dt.float32r,
                          kind="ExternalInput").ap()
    out_d = nc.dram_tensor("out", [1, NE * BC * (NB + 1)], F32,
                           kind="ExternalOutput").ap()

    P = ctx.enter_context  # pools

    const = P(tc.tile_pool(name="const", bufs=1))
    xtp = P(tc.tile_pool(name="xt", bufs=6))
    psE = P(tc.tile_pool(name="psE", bufs=2, space="PSUM"))
    psS = P(tc.tile_pool(name="psS", bufs=1, space="PSUM"))
    small = P(tc.tile_pool(name="small", bufs=2))

    # ---- persistent SBUF ----
    W_sb = const.tile([128, NCHUNK * E], BF16)
    nc.sync.dma_start(W_sb[:], Wemb[:])
    bemb_sb = const.tile([E, 1], F32)
    nc.scalar.dma_start(bemb_sb[:], bemb[:])
    WihT_sb = const.tile([E, 300], F32)
    nc.scalar.dma_start(WihT_sb[:], WihT[:])
    WhhT_sb = const.tile([E, 300], F32)
    nc.scalar.dma_start(WhhT_sb[:], WhhT[:])
    bias4_sb = const.tile([E, 4], F32)
    nc.scalar.dma_start(bias4_sb[:], bias4[:])
    Wbil_sb = const.tile([E, NE * E], F32)
    nc.scalar.dma_start(Wbil_sb[:], Wbil[:])
    ones_sb = const.tile([E, 1], mybir.dt.float32r)
    nc.scalar.dma_start(ones_sb[:], ones[:])

    ET = const.tile([E, ROWS], F32)                # all embeddings, transposed
    gi_sb = const.tile([E, NE * 3 * BC], F32)      # preacts, [s][r|z|n] blocks
    h = const.tile([E, BC], F32)                   # GRU hidden state (h^T)
    tmp_all = const.tile([E, NE * BC * (NB + 1)], mybir.dt.float32r)
    out_sb = const.tile([1, NE * BC * (NB + 1)], F32)

    A_sb = const.tile([E, NE * BC], F32)           # bilinear A, persisted
    gi_v = gi_sb.rearrange("e (s g b) -> e s g b", s=NE, g=3)
    tmp_v = tmp_all.rearrange("e (s b p) -> e s b p", s=NE, b=BC)
    Eb_v = ET[:, 512:ROWS].rearrange("e (nb s b) -> e nb s b", nb=NB, s=NE)

    def bil_part(s0, s1):
        # A_s = W_bil[s].T @ h^T plus the Ep and nb0-5 score products
        # (blocks 0-3, all evacuated by end of pass 1) -- spread over
        # pass 2; the nb6-9 products run as the tail.
        for s in range(s0, s1):
            Ap = psS.tile([E, BC], F32, tag="bilA", name="Ap", bufs=2)
            nc.tensor.matmul(Ap[:, :], Wbil_sb[:, s * E:(s + 1) * E], h[:],
                             start=True, stop=True)
            nc.scalar.copy(A_sb[:, s * BC:(s + 1) * BC], Ap[:])
            nc.vector.tensor_mul(tmp_v[:, s, :, 0],
                                 ET[:, NE * BC + s * BC: NE * BC + (s + 1) * BC],
                                 Ap[:])
            nc.vector.tensor_mul(
                tmp_v[:, s, :, 1:7].rearrange("e b p -> e p b"),
                Eb_v[:, 0:6, s, :],
                Ap[:].unsqueeze(1).broadcast_to([E, 6, BC]))

    def gru_init():
        nc.vector.memset(h[:], 0.0)
        for g in range(3):
            gp = psS.tile([E, NE * BC], F32, tag="sp0", name="gp")
            nc.tensor.matmul(gp[:, :], WihT_sb[:, g * E:(g + 1) * E],
                             ET[:, 0:NE * BC], start=True, stop=True)
            nc.scalar.add(gi_v[:, :, g, :],
                          gp.rearrange("e (s b) -> e s b", s=NE),
                          bias4_sb[:, g:g + 1])

    def gru_step(s):
        # serial chain on DVE (elementwise) + ACT (sigmoid/tanh) only;
        # gpsimd stays free for DMA issue so the chain never stalls
        # behind buffer-full DMA waits
        c0 = s * 3 * BC
        gh = psS.tile([E, 3 * BC], F32, tag="sp1", name="gh")
        for g in range(3):
            nc.tensor.matmul(gh[:, g * BC:(g + 1) * BC],
                             WhhT_sb[:, g * E:(g + 1) * E], h[:],
                             start=True, stop=True)
        rzt = small.tile([E, 2 * BC], F32, tag="rzt", name="rzt")
        nc.vector.tensor_add(rzt[:], gh[:, 0:2 * BC], gi_sb[:, c0:c0 + 2 * BC])
        rz = small.tile([E, 2 * BC], F32, tag="rz", name="rz")
        nc.scalar.activation(rz[:], rzt[:],
                             mybir.ActivationFunctionType.Sigmoid)
        hn = small.tile([E, BC], F32, tag="hn", name="hn")
        nc.vector.tensor_scalar_add(hn[:], gh[:, 2 * BC:3 * BC],
                                    bias4_sb[:, 3:4])  # gh_n + b_hn
        t1 = small.tile([E, BC], F32, tag="t1", name="t1")
        nc.vector.tensor_mul(t1[:], rz[:, 0:BC], hn[:])
        t2 = small.tile([E, BC], F32, tag="t2", name="t2")
        nc.vector.tensor_add(t2[:], t1[:], gi_sb[:, c0 + 2 * BC:c0 + 3 * BC])
        n = small.tile([E, BC], F32, tag="n", name="n")
        nc.scalar.activation(n[:], t2[:], mybir.ActivationFunctionType.Tanh)
        d = small.tile([E, BC], F32, tag="d", name="d")
        nc.vector.tensor_sub(d[:], h[:], n[:])
        zd = small.tile([E, BC], F32, tag="zd", name="zd")
        nc.vector.tensor_mul(zd[:], rz[:, BC:2 * BC], d[:])
        nc.vector.tensor_add(h[:], n[:], zd[:])    # h = n + z*(h-n)

    # ---- embed: 3 passes over k, each a 1024-column block-pair ----
    for p in range(3):
        et0 = psE.tile([E, 512], F32, tag="et0", name="et0")
        et1 = psE.tile([E, 512], F32, tag="et1", name="et1")
        c0 = p * 1024
        for jt in range(NT):
            j0 = jt * 4
            jn = min(4, NCHUNK - j0)
            # interleaved serial work: GRU in pass 1, bilinear in pass 2
            if p == 1 and jt < NE:
                gru_step(jt)
            elif p == 2 and jt % 4 == 0 and jt < 16:
                q = jt // 4
                bil_part(4 * q, 4 * q + 4)
            xt = xtp.tile([128, 4096], BF16, name="xt")
            eng = nc.gpsimd if jt % 2 == 0 else nc.sync
            eng.dma_start(xt[:, 0:jn * 1024],
                          XTs[p][:, j0 * 1024:(j0 + jn) * 1024])
            for u in range(jn):
                j = j0 + u
                nc.tensor.matmul(et0[:, :], W_sb[:, j * E:(j + 1) * E],
                                 xt[:, u * 1024:u * 1024 + 512],
                                 start=(j == 0), stop=(j == NCHUNK - 1),
                                 skip_group_check=True)
                nc.tensor.matmul(et1[:, :], W_sb[:, j * E:(j + 1) * E],
                                 xt[:, u * 1024 + 512:(u + 1) * 1024],
                                 start=(j == 0), stop=(j == NCHUNK - 1),
                                 skip_group_check=True)
        nc.scalar.add(ET[:, c0:c0 + 512], et0[:, :], bemb_sb[:, 0:1])
        nc.scalar.add(ET[:, c0 + 512:c0 + 1024], et1[:, :], bemb_sb[:, 0:1])
        # gi preacts as soon as block 0 (Ec) is done
        if p == 0:
            gru_init()

    # ---- tail: nb6-9 products (blocks 4-5) + ones-matmul reduction ----
    for s in range(NE):
        nc.vector.tensor_mul(
            tmp_v[:, s, :, 7:NB + 1].rearrange("e b p -> e p b"),
            Eb_v[:, 6:10, s, :],
            A_sb[:, s * BC:(s + 1) * BC].unsqueeze(1).broadcast_to([E, 4, BC]))
    TOT = NE * BC * (NB + 1)
    for cc in range(0, TOT, 512):
        w = min(512, TOT - cc)
        rp = psS.tile([1, 512], F32, tag="sp1")
        nc.tensor.matmul(rp[0:1, 0:w], ones_sb[:, 0:1], tmp_all[:, cc:cc + w],
                         start=True, stop=True)
        nc.scalar.copy(out_sb[:, cc:cc + w], rp[0:1, 0:w])
    nc.sync.dma_start(out_d[:], out_sb[:])


def build():
    import contextlib
    nc = bacc.Bacc("TRN2", target_bir_lowering=False, debug=False,
                   enable_asserts=False, num_devices=N_CORES)
    with tile.TileContext(nc) as tc:
        with contextlib.ExitStack() as ctx:
            _emit(nc, tc, ctx)
    nc.compile()
    return nc


_NC = None


def make_in_maps(Xc, Xp, Xb, W_embed, b_embed, W_ih, W_hh, b_ih, b_hh, W_bil):
    import ml_dtypes
    B = Xc.shape[0]
    BF = ml_dtypes.bfloat16
    Xc_b = np.asarray(Xc, np.float32).reshape(B, NE, CT).astype(BF)
    Xp_b = np.asarray(Xp, np.float32).reshape(B, NE, CT).astype(BF)
    Xb_b = np.asarray(Xb, np.float32).reshape(B, NE, NB, CT).astype(BF)

    W_embed = np.ascontiguousarray(W_embed, np.float32)
    W_ch = np.zeros((128, NCHUNK * E), np.float32)
    for j in range(NCHUNK):
        kj = min(128, CT - j * 128)
        W_ch[:kj, j * E:(j + 1) * E] = W_embed[j * 128:j * 128 + kj]
    W_ch = W_ch.astype(BF)
    bemb = np.ascontiguousarray(b_embed, np.float32).reshape(E, 1)
    WihT = np.ascontiguousarray(W_ih.T, np.float32)          # [100, 300]
    WhhT = np.ascontiguousarray(W_hh.T, np.float32)
    bias4 = np.stack([b_ih[0:E] + b_hh[0:E],
                      b_ih[E:2 * E] + b_hh[E:2 * E],
                      b_ih[2 * E:3 * E],
                      b_hh[2 * E:3 * E]], axis=1).astype(np.float32)
    Wbil_r = np.ascontiguousarray(
        np.transpose(W_bil, (1, 0, 2)).reshape(E, NE * E), np.float32)
    ones = np.ones((E, 1), np.float32)

    shared = dict(Wemb=W_ch, bemb=bemb, WihT=WihT, WhhT=WhhT,
                  bias4=bias4, Wbil=Wbil_r, ones=ones)
    in_maps = []
    for c in range(N_CORES):
        sl = slice(c * BC, (c + 1) * BC)
        # rows in ET column order: Xc (s,b) | Xp (s,b) | Xb (nb,s,b)
        A = np.zeros((ROWS, CTP), BF)
        A[0:256, 0:CT] = Xc_b[sl].transpose(1, 0, 2).reshape(256, CT)
        A[256:512, 0:CT] = Xp_b[sl].transpose(1, 0, 2).reshape(256, CT)
        A[512:, 0:CT] = Xb_b[sl].transpose(2, 1, 0, 3).reshape(2560, CT)
        m = dict(shared)
        for p in range(3):
            m[f"XT{p}"] = np.ascontiguousarray(
                A[p * 1024:(p + 1) * 1024]
                .reshape(1024, NCHUNK, 128)
                .transpose(2, 1, 0)
                .reshape(128, NCHUNK * 1024))
        in_maps.append(m)
    return in_maps


def gather(results):
    outs = []
    for c in range(N_CORES):
        o = results[c]["out"].reshape(NE, BC, NB + 1)       # [s, b, p]
        outs.append(np.transpose(o, (1, 0, 2)))             # [b, s, p]
    return np.concatenate(outs, axis=0).astype(np.float32)  # [128, 16, 11]


def kernel(Xc, Xp, Xb, W_embed, b_embed, W_ih, W_hh, b_ih, b_hh, W_bil):
    global _NC
    if _NC is None:
        _NC = build()
    in_maps = make_in_maps(Xc, Xp, Xb, W_embed, b_embed, W_ih, W_hh,
                           b_ih, b_hh, W_bil)
    res = run_bass_kernel_spmd(_NC, in_maps, core_ids=list(range(N_CORES)))
    return gather(res.results)
